# revision 1
# baseline (speedup 1.0000x reference)
"""Invariant Point Attention kernel for Trainium2, 8-core SPMD.

Strategy: sequence-parallel over the query axis n (96 rows/core). Each core
computes full k/v/k_pts from `single` (replicated, tiny), its own q rows, its
own [96, 768] slice of the pair tensor (host-transposed to [PC, n, m] so the
PC=128 contraction lands on SBUF partitions), full attention rows (softmax
over m is core-local -> zero collectives), and its [96, 384] output slice.

Math notes vs the reference:
  - terms constant along the softmax axis m cancel exactly (q2, bk, bpb) and
    are dropped;
  - SCALE is folded into Wq/bq, Wqp/bqp and the q-side trans on the host;
  - softmax runs without max-subtraction (logits are O(10), exp is safe in
    fp32); the denominator is applied after the attn@v matmul by linearity.
"""

import sys

for p in ("/opt/trn_rl_repo", "/opt/trn_rl_repo/concourse"):
    if p not in sys.path:
        sys.path.append(p)

import numpy as np

import concourse.bass as bass
import concourse.tile as tile
from concourse import bacc, mybir
from concourse.bass_utils import run_bass_kernel_spmd

F32 = mybir.dt.float32
AX = mybir.AxisListType
ALU = mybir.AluOpType
ACTF = mybir.ActivationFunctionType

B, N, C, PC, H, P = 1, 768, 384, 128, 12, 3
Ch = C // H            # 32
HD = H * P * P         # 108
SCALE = Ch ** -0.5
EPS = 1e-5
NCORES = 8
NO = N // NCORES       # 96 own query rows per core
NG = 4                 # n rows per pair-phase group
GROUPS = NO // NG      # 12
MT = N // 128          # 6 m tiles


def _build(nc):
    dt_ = lambda name, shape: nc.dram_tensor(name, shape, F32, kind="ExternalInput").ap()
    singleT = dt_("singleT", [C, N])
    sTo = dt_("sTo", [C, NO])
    so = dt_("so", [NO, C])
    pairT = dt_("pairT", [PC, NO, N])
    rot9 = dt_("rot9", [N, 9])
    roto = dt_("roto", [NO, 9])
    trans3 = dt_("trans3", [N, 3])
    transqo = dt_("transqo", [NO, 3])
    Wq = dt_("Wq", [C, 512])
    bq = dt_("bq", [1, 512])
    Wk = dt_("Wk", [C, 512])
    Wv = dt_("Wv", [C, C])
    bv = dt_("bv", [1, C])
    Wqp = dt_("Wqp", [C, HD])
    bqp = dt_("bqp", [1, HD])
    Wkp = dt_("Wkp", [C, HD])
    bkp = dt_("bkp", [1, HD])
    Wpb = dt_("Wpb", [PC, H])
    Wo = dt_("Wo", [C, C])
    bo = dt_("bo", [1, C])
    gamB = dt_("gamB", [NO, C])
    betB = dt_("betB", [NO, C])
    id128 = dt_("id128", [128, 128])
    out = nc.dram_tensor("out", [NO, C], F32, kind="ExternalOutput").ap()

    with tile.TileContext(nc) as tc:
        _kernel(tc, locals())
    return out


def _kernel(tc, t):
    nc = tc.nc
    mm = nc.tensor.matmul
    dma = nc.sync.dma_start

    const = tc.alloc_tile_pool(name="const", bufs=1)
    big = tc.alloc_tile_pool(name="big", bufs=1)

    # ---- load constants / weights ----
    def load(name, shape, src):
        tl = const.tile(list(shape), F32, tag=name)
        dma(tl[:], src)
        return tl

    Wq_sb = const.tile([128, 3 * 512], F32, tag="Wq_sb")
    Wk_sb = const.tile([128, 3 * 512], F32, tag="Wk_sb")
    for W_sb, name in ((Wq_sb, "Wq"), (Wk_sb, "Wk")):
        for tt in range(3):
            dma(W_sb[:, tt * 512:(tt + 1) * 512], t[name][tt * 128:(tt + 1) * 128, :])
    Wv_sb = const.tile([128, 3 * C], F32, tag="Wv_sb")
    Wo_sb = const.tile([128, 3 * C], F32, tag="Wo_sb")
    for W_sb, name in ((Wv_sb, "Wv"), (Wo_sb, "Wo")):
        for tt in range(3):
            dma(W_sb[:, tt * C:(tt + 1) * C], t[name][tt * 128:(tt + 1) * 128, :])
    Wqp_sb = const.tile([128, 3 * HD], F32, tag="Wqp_sb")
    Wkp_sb = const.tile([128, 3 * HD], F32, tag="Wkp_sb")
    for W_sb, name in ((Wqp_sb, "Wqp"), (Wkp_sb, "Wkp")):
        for tt in range(3):
            dma(W_sb[:, tt * HD:(tt + 1) * HD], t[name][tt * 128:(tt + 1) * 128, :])
    Wpb_sb = load("Wpb_sb", (PC, H), t["Wpb"])
    bq_sb = load("bq_sb", (1, 512), t["bq"])
    bv_sb = load("bv_sb", (1, C), t["bv"])
    bqp_sb = load("bqp_sb", (1, HD), t["bqp"])
    bkp_sb = load("bkp_sb", (1, HD), t["bkp"])
    bo_sb = load("bo_sb", (1, C), t["bo"])
    id_sb = load("id_sb", (128, 128), t["id128"])
    gam_sb = load("gam_sb", (NO, C), t["gamB"])
    bet_sb = load("bet_sb", (NO, C), t["betB"])
    so_sb = load("so_sb", (NO, C), t["so"])
    roto_sb = load("roto_sb", (NO, 9), t["roto"])
    transqo_sb = load("transqo_sb", (NO, 3), t["transqo"])
    sT_sb = const.tile([128, 3 * N], F32, tag="sT_sb")
    for tt in range(3):
        dma(sT_sb[:, tt * N:(tt + 1) * N], t["singleT"][tt * 128:(tt + 1) * 128, :])
    sTo_sb = const.tile([128, 3 * NO], F32, tag="sTo_sb")
    for tt in range(3):
        dma(sTo_sb[:, tt * NO:(tt + 1) * NO], t["sTo"][tt * 128:(tt + 1) * 128, :])
    rot_sb = const.tile([128, 6 * 9], F32, tag="rot_sb")
    trans_sb = const.tile([128, 6 * 3], F32, tag="trans_sb")
    for mt in range(MT):
        dma(rot_sb[:, mt * 9:(mt + 1) * 9], t["rot9"][mt * 128:(mt + 1) * 128, :])
        dma(trans_sb[:, mt * 3:(mt + 1) * 3], t["trans3"][mt * 128:(mt + 1) * 128, :])
    ones_col = const.tile([128, 1], F32, tag="ones_col")
    nc.vector.memset(ones_col[:], 1.0)
    ones96 = const.tile([1, NO], F32, tag="ones96")
    nc.vector.memset(ones96[:], 1.0)
    ones128 = const.tile([1, 128], F32, tag="ones128")
    nc.vector.memset(ones128[:], 1.0)

    # ---- big persistent sbuf ----
    kT_sb = big.tile([128, 4 * N], F32, tag="kT")        # [c_out, m] (3 row-tiles)
    qT_sb = big.tile([128, 4 * NO], F32, tag="qT")       # [c_out, n]
    v_sb = big.tile([128, MT * C], F32, tag="v")         # per m-tile [128, 384]
    qg_sb = big.tile([NO, HD], F32, tag="qg")
    kg_sb = big.tile([128, MT * HD], F32, tag="kg")
    # per-head transposed points, head h at partitions 32*(h%4), col block h//4
    qgT_sb = big.tile([128, 4 * NO], F32, tag="qgT")
    kgT_sb = big.tile([128, 4 * N], F32, tag="kgT")
    k2s_sb = big.tile([128, MT * H], F32, tag="k2s")     # -0.5*SCALE*k2, per m-tile
    pb_sb = big.tile([128, MT * H * NO], F32, tag="pb")  # pair bias [m | (h, n)]
    E_sb = big.tile([128, MT * H * NO], F32, tag="E")    # exp(logits) [m | (h, n)]

    with tc.tile_pool(name="pro", bufs=3, space="PSUM") as pro, \
         tc.tile_pool(name="work", bufs=6) as work:

        # ---- kT = (single @ Wk)^T : [c_out, m], no bias (cancels in softmax)
        for j in range(4):
            for half in range(2):
                ps = pro.tile([128, 384], F32, tag="ps")
                for tt in range(3):
                    mm(ps[:], Wk_sb[:, tt * 512 + j * 128: tt * 512 + (j + 1) * 128],
                       sT_sb[:, tt * N + half * 384: tt * N + (half + 1) * 384],
                       start=(tt == 0), stop=(tt == 2))
                nc.vector.tensor_copy(kT_sb[:, j * N + half * 384: j * N + (half + 1) * 384], ps[:])

        # ---- qT = (single_own @ (SCALE*Wq))^T + SCALE*bq : [c_out, n]
        for j in range(4):
            ps = pro.tile([128, NO], F32, tag="ps")
            for tt in range(3):
                mm(ps[:], Wq_sb[:, tt * 512 + j * 128: tt * 512 + (j + 1) * 128],
                   sTo_sb[:, tt * NO:(tt + 1) * NO], start=(tt == 0), stop=False)
            mm(ps[:], bq_sb[0:1, j * 128:(j + 1) * 128], ones96[:], start=False, stop=True)
            nc.vector.tensor_copy(qT_sb[:, j * NO:(j + 1) * NO], ps[:])

        # ---- v = single @ Wv + bv : [m, c_out]
        for mt in range(MT):
            ps = pro.tile([128, 384], F32, tag="ps")
            for tt in range(3):
                mm(ps[:], sT_sb[:, tt * N + mt * 128: tt * N + (mt + 1) * 128],
                   Wv_sb[:, tt * C:(tt + 1) * C], start=(tt == 0), stop=False)
            mm(ps[:], ones128[:], bv_sb[:], start=False, stop=True)
            nc.vector.tensor_copy(v_sb[:, mt * C:(mt + 1) * C], ps[:])

        # ---- point projections: qp [n, 108] (SCALE folded), kp per m-tile
        qp_sb = work.tile([NO, HD], F32, tag="qp")
        ps = pro.tile([128, 384], F32, tag="ps")
        for tt in range(3):
            mm(ps[:NO, :HD], sTo_sb[:, tt * NO:(tt + 1) * NO],
               Wqp_sb[:, tt * HD:(tt + 1) * HD], start=(tt == 0), stop=False)
        mm(ps[:NO, :HD], ones96[:], bqp_sb[:], start=False, stop=True)
        nc.vector.tensor_copy(qp_sb[:], ps[:NO, :HD])

        kp_tiles = []
        for mt in range(MT):
            ps = pro.tile([128, 384], F32, tag="ps")
            for tt in range(3):
                mm(ps[:, :HD], sT_sb[:, tt * N + mt * 128: tt * N + (mt + 1) * 128],
                   Wkp_sb[:, tt * HD:(tt + 1) * HD], start=(tt == 0), stop=False)
            mm(ps[:, :HD], ones128[:], bkp_sb[:], start=False, stop=True)
            kp = work.tile([128, HD], F32, tag="kp")
            nc.vector.tensor_copy(kp[:], ps[:, :HD])
            kp_tiles.append(kp)

        # ---- rotations: g[n,h,d,j] = sum_i p[n,h,d,i]*rot[n,3i+j] (+ trans[n,d])
        def rotate(dst, src, rsb, roff, tsb, toff, rows):
            dv = dst.rearrange("p (h d j) -> p h d j", d=3, j=3)
            sv = src.rearrange("p (h d i) -> p h d i", d=3, i=3)
            for j in range(3):
                acc = work.tile([rows, 36], F32, tag="rotacc")
                av = acc[:].rearrange("p (h d) -> p h d", d=3)
                nc.vector.tensor_scalar_mul(av, sv[:, :, :, 0], rsb[:rows, roff + j: roff + j + 1])
                for i in (1, 2):
                    nc.vector.scalar_tensor_tensor(
                        av, sv[:, :, :, i], rsb[:rows, roff + 3 * i + j: roff + 3 * i + j + 1],
                        av, op0=ALU.mult, op1=ALU.add)
                nc.vector.tensor_copy(dv[:, :, :, j], av)
            for d in range(3):
                nc.vector.tensor_scalar_add(dv[:, :, d, :], dv[:, :, d, :],
                                            tsb[:rows, toff + d: toff + d + 1])

        rotate(qg_sb[:], qp_sb[:], roto_sb, 0, transqo_sb, 0, NO)
        for mt in range(MT):
            rotate(kg_sb[:, mt * HD:(mt + 1) * HD], kp_tiles[mt][:],
                   rot_sb, mt * 9, trans_sb, mt * 3, 128)

        # ---- k2s = -0.5*SCALE*sum_dj kg^2 : [m, h] per m-tile
        for mt in range(MT):
            sq = work.tile([128, HD], F32, tag="sq")
            kgs = kg_sb[:, mt * HD:(mt + 1) * HD]
            nc.vector.tensor_mul(sq[:], kgs, kgs)
            red = work.tile([128, H], F32, tag="red")
            nc.vector.tensor_reduce(red[:], sq[:].rearrange("p (h e) -> p h e", e=9),
                                    axis=AX.X, op=ALU.add)
            nc.vector.tensor_scalar_mul(k2s_sb[:, mt * H:(mt + 1) * H], red[:], -0.5 * SCALE)

        # ---- transpose qg, kg -> per-head [(d,j)=9 rows @ 32*(h%4), n/m]
        for h in range(H):
            bp, blk = 32 * (h % 3), h // 3
            ps = pro.tile([128, 384], F32, tag="ps")
            mm(ps[bp:bp + 9, :NO], qg_sb[:, h * 9:(h + 1) * 9],
               id_sb[:NO, :NO], start=True, stop=True)
            nc.vector.tensor_copy(qgT_sb[bp:bp + 9, blk * NO:(blk + 1) * NO],
                                  ps[bp:bp + 9, :NO])
        for mt in range(MT):
            for h in range(H):
                bp, blk = 32 * (h % 3), h // 3
                ps = pro.tile([128, 384], F32, tag="ps")
                mm(ps[bp:bp + 9, :128],
                   kg_sb[:, mt * HD + h * 9: mt * HD + (h + 1) * 9],
                   id_sb[:], start=True, stop=True)
                nc.vector.tensor_copy(
                    kgT_sb[bp:bp + 9, blk * N + mt * 128: blk * N + (mt + 1) * 128],
                    ps[bp:bp + 9, :128])

        # ---- pair phase: pb tile stationary -> out [m, h] directly
        with tc.tile_pool(name="pp", bufs=4, space="PSUM") as pp, \
             tc.tile_pool(name="pairp", bufs=2) as pairp:
            for g in range(GROUPS):
                pg = pairp.tile([128, NG * N], F32, tag="pg")
                dma(pg[:], t["pairT"][:, g * NG:(g + 1) * NG, :])
                for mt in range(MT):
                    ps = pp.tile([128, NG * H], F32, tag="ps")
                    for ns in range(NG):
                        mm(ps[:, ns * H:(ns + 1) * H],
                           pg[:, ns * N + mt * 128: ns * N + (mt + 1) * 128],
                           Wpb_sb[:], start=True, stop=True)
                    # ps is [m, (n8, h)]; scatter into pb_sb [m, (h, n)]
                    dst = pb_sb[:, mt * H * NO:(mt + 1) * H * NO] \
                        .rearrange("p (h n) -> p h n", h=H)[:, :, g * NG:(g + 1) * NG] \
                        .transpose([0, 2, 1])
                    nc.vector.tensor_copy(dst, ps[:].rearrange("p (n h) -> p n h", h=H))

    # ---- attention ----
    with tc.tile_pool(name="pL", bufs=4, space="PSUM") as pL, \
         tc.tile_pool(name="pacc", bufs=1, space="PSUM") as pacc, \
         tc.tile_pool(name="att", bufs=3) as att:
        av_ps = pacc.tile([NO, C], F32, tag="av")
        dn_ps = pacc.tile([NO, H], F32, tag="dn")
        for mt in range(MT):
            tmp = att.tile([128, H * NO], F32, tag="tmp")
            for h in range(H):
                L = pL.tile([128, NO], F32, tag="L")
                tl, tr = h // 3, 32 * (h % 3)
                mm(L[:], kT_sb[tr:tr + 32, tl * N + mt * 128: tl * N + (mt + 1) * 128],
                   qT_sb[tr:tr + 32, tl * NO:(tl + 1) * NO], start=True, stop=False)
                mm(L[:], kgT_sb[tr:tr + 9, tl * N + mt * 128: tl * N + (mt + 1) * 128],
                   qgT_sb[tr:tr + 9, tl * NO:(tl + 1) * NO], start=False, stop=True)
                nc.vector.scalar_tensor_tensor(
                    tmp[:, h * NO:(h + 1) * NO], L[:], k2s_sb[:, mt * H + h: mt * H + h + 1],
                    pb_sb[:, (mt * H + h) * NO:(mt * H + h + 1) * NO],
                    op0=ALU.add, op1=ALU.add)
            eslab = E_sb[:, mt * H * NO:(mt + 1) * H * NO]
            nc.scalar.activation(eslab, tmp[:], ACTF.Exp)
        for h in range(H):
            for mt in range(MT):
                e = E_sb[:, (mt * H + h) * NO:(mt * H + h + 1) * NO]
                mm(av_ps[:, h * Ch:(h + 1) * Ch], e,
                   v_sb[:, mt * C + h * Ch: mt * C + (h + 1) * Ch],
                   start=(mt == 0), stop=(mt == MT - 1))
            for mt in range(MT):
                e = E_sb[:, (mt * H + h) * NO:(mt * H + h + 1) * NO]
                mm(dn_ps[:, h:h + 1], e, ones_col[:], start=(mt == 0), stop=(mt == MT - 1))

        # ---- epilogue: divide, out-proj, residual, layernorm ----
        rcp = att.tile([NO, H], F32, tag="rcp")
        nc.vector.reciprocal(rcp[:], dn_ps[:])
        w_sb = att.tile([NO, C], F32, tag="w")
        for h in range(H):
            nc.vector.tensor_scalar_mul(w_sb[:, h * Ch:(h + 1) * Ch],
                                        av_ps[:, h * Ch:(h + 1) * Ch], rcp[:, h:h + 1])
        wT_sb = att.tile([128, 3 * NO], F32, tag="wT")
        for tt in range(3):
            tp = pL.tile([128, NO], F32, tag="L")
            nc.tensor.transpose(tp[:], w_sb[:, tt * 128:(tt + 1) * 128], id_sb[:NO, :NO])
            nc.vector.tensor_copy(wT_sb[:, tt * NO:(tt + 1) * NO], tp[:])
        o_ps = pacc.tile([NO, C], F32, tag="av")
        for tt in range(3):
            mm(o_ps[:], wT_sb[:, tt * NO:(tt + 1) * NO], Wo_sb[:, tt * C:(tt + 1) * C],
               start=(tt == 0), stop=False)
        mm(o_ps[:], ones96[:], bo_sb[:], start=False, stop=True)
        x_sb = att.tile([NO, C], F32, tag="x")
        nc.vector.tensor_add(x_sb[:], o_ps[:], so_sb[:])
        mu = att.tile([NO, 1], F32, tag="mu")
        nc.vector.tensor_reduce(mu[:], x_sb[:], axis=AX.X, op=ALU.add)
        nc.vector.tensor_scalar_mul(mu[:], mu[:], 1.0 / C)
        xm = att.tile([NO, C], F32, tag="xm")
        nc.vector.tensor_scalar_sub(xm[:], x_sb[:], mu[:])
        sq = att.tile([NO, C], F32, tag="sqe")
        nc.vector.tensor_mul(sq[:], xm[:], xm[:])
        var = att.tile([NO, 1], F32, tag="var")
        nc.vector.tensor_reduce(var[:], sq[:], axis=AX.X, op=ALU.add)
        epsb = att.tile([NO, 1], F32, tag="epsb")
        nc.vector.memset(epsb[:], EPS)
        std = att.tile([NO, 1], F32, tag="std")
        nc.scalar.activation(std[:], var[:], ACTF.Sqrt, bias=epsb[:], scale=1.0 / C)
        rstd = att.tile([NO, 1], F32, tag="rstd")
        nc.vector.reciprocal(rstd[:], std[:])
        y = att.tile([NO, C], F32, tag="y")
        nc.vector.tensor_scalar_mul(y[:], xm[:], rstd[:])
        nc.vector.tensor_mul(y[:], y[:], gam_sb[:])
        nc.vector.tensor_add(y[:], y[:], bet_sb[:])
        dma(t["out"], y[:])
    big.release()
    const.release()


_CACHE = {}


def _get_program():
    if "nc" not in _CACHE:
        nc = bacc.Bacc("TRN2", target_bir_lowering=False, debug=False,
                       num_devices=NCORES)
        _build(nc)
        nc.compile()
        _CACHE["nc"] = nc
    return _CACHE["nc"]


def _pad_heads(W):
    # scatter head h (32 cols) to col 128*(h//3) + 32*(h%3) of a 512-wide buffer
    out = np.zeros(W.shape[:-1] + (512,), np.float32)
    for h in range(H):
        out[..., 128 * (h // 3) + 32 * (h % 3): 128 * (h // 3) + 32 * (h % 3) + Ch] = \
            W[..., h * Ch:(h + 1) * Ch]
    return out


def make_in_maps(single, pair, rot, trans, Wq, bq, Wk, bk, Wv, bv, Wpb, bpb,
                 Wqp, bqp, Wkp, bkp, Wo, bo, gamma, beta):
    f = lambda a: np.ascontiguousarray(np.asarray(a), dtype=np.float32)
    s = f(single)[0]
    sT = f(s.T)
    common = {
        "singleT": sT,
        "rot9": f(rot)[0].reshape(N, 9),
        "trans3": f(trans)[0],
        "Wq": _pad_heads(f(Wq) * SCALE), "bq": _pad_heads((f(bq) * SCALE).reshape(1, C)),
        "Wk": _pad_heads(f(Wk)),
        "Wv": f(Wv), "bv": f(bv).reshape(1, C),
        "Wqp": f(Wqp) * SCALE, "bqp": (f(bqp) * SCALE).reshape(1, HD),
        "Wkp": f(Wkp), "bkp": f(bkp).reshape(1, HD),
        "Wpb": f(Wpb),
        "Wo": f(Wo), "bo": f(bo).reshape(1, C),
        "gamB": np.ascontiguousarray(np.broadcast_to(f(gamma), (NO, C))),
        "betB": np.ascontiguousarray(np.broadcast_to(f(beta), (NO, C))),
        "id128": np.eye(128, dtype=np.float32),
    }
    pr = f(pair)[0]
    tr = f(trans)[0] * SCALE
    ro = f(rot)[0].reshape(N, 9)
    in_maps = []
    for c in range(NCORES):
        lo, hi = c * NO, (c + 1) * NO
        m = dict(common)
        m["sTo"] = np.ascontiguousarray(s[lo:hi].T)
        m["so"] = np.ascontiguousarray(s[lo:hi])
        m["pairT"] = np.ascontiguousarray(pr[lo:hi].transpose(2, 0, 1))
        m["roto"] = np.ascontiguousarray(ro[lo:hi])
        m["transqo"] = np.ascontiguousarray(tr[lo:hi])
        in_maps.append(m)
    return in_maps


def run(in_maps, **kwargs):
    nc = _get_program()
    return run_bass_kernel_spmd(nc, in_maps, core_ids=list(range(NCORES)), **kwargs)


def kernel(**inputs):
    res = run(make_in_maps(**inputs))
    out = np.concatenate([res.results[c]["out"] for c in range(NCORES)], axis=0)
    return out.reshape(B, N, C).astype(np.float32)



# revision 22
# speedup vs baseline: 2.9415x; 2.9415x over previous
"""Invariant Point Attention kernel for Trainium2, 8-core SPMD.

Strategy: sequence-parallel over the query axis n (96 rows/core), m-major
pipeline over 6 key-tiles of 128. Each core computes full k/v/k_pts from
`single` (replicated, tiny), its own q rows, and streams its [96, 768, 128]
pair slice (host-transposed to [PC, m-group, n, m] fp8) group by group,
overlapping DMA with PE work.

Perf notes vs the fp32 baseline (460us):
  - all matmul operands bf16 (fp8 for pair) -> FWL weight loads + 1
    cycle/row streaming instead of fp32's 4;
  - pair GEMM is weight-load-bound (128-col lhsT per 12-col output);
    fp8 weights halve the LD again and halve the 37.7MB/core DMA twice;
  - softmax denominator fused into the attn@v matmul via ones-columns
    interleaved in v (kills 144 MATMUL 128*96*1 + their weight loads);
  - m-major order lets E[mt] be consumed immediately -> small slabs.

Math notes vs the reference:
  - terms constant along the softmax axis m cancel exactly (q2, bk, bpb)
    and are dropped;
  - SCALE is folded into Wq/bq, Wqp/bqp and the q-side trans on the host;
  - Wpb is scaled x64 on the host to stay out of fp8 subnormals; the pair
    psum copy multiplies by 1/64;
  - softmax runs without max-subtraction (logits are O(10), exp is safe in
    fp32); the denominator is applied after the attn@v matmul by linearity;
  - rotation + k2 stay fp32 (DVE); only matmul operands are quantized.
"""

import sys

for p in ("/opt/trn_rl_repo", "/opt/trn_rl_repo/concourse"):
    if p not in sys.path:
        sys.path.append(p)

import os
import numpy as np
import ml_dtypes

STAGE = os.environ.get("K_STAGE", "full")  # proj | pair | full

import concourse.bass as bass
import concourse.tile as tile
from concourse import bacc, mybir
from concourse.bass_utils import run_bass_kernel_spmd

F32 = mybir.dt.float32
BF16 = mybir.dt.bfloat16
FP8 = mybir.dt.float8e4
AX = mybir.AxisListType
ALU = mybir.AluOpType
ACTF = mybir.ActivationFunctionType

B, N, C, PC, H, P = 1, 768, 384, 128, 12, 3
Ch = C // H            # 32
HD = H * P * P         # 108
SCALE = Ch ** -0.5
EPS = 1e-5
NCORES = 8
NO = N // NCORES       # 96 own query rows per core
MT = N // 128          # 6 m-groups
GSZ = NO * 128         # pair elems per partition per m-group
WPB_SCALE = 64.0       # fp8 subnormal dodge for Wpb


def _build(nc):
    def dt_(name, shape, dt=F32):
        return nc.dram_tensor(name, shape, dt, kind="ExternalInput").ap()

    singleT = dt_("singleT", [C, N], BF16)
    sTo = dt_("sTo", [C, NO], BF16)
    so = dt_("so", [NO, C])
    pairT = dt_("pairT", [PC, MT * GSZ], FP8)
    rot9 = dt_("rot9", [N, 9])
    roto = dt_("roto", [NO, 9])
    trans3 = dt_("trans3", [N, 3])
    transqo = dt_("transqo", [NO, 3])
    Wq = dt_("Wq", [C, 512], BF16)
    bq = dt_("bq", [1, 512], BF16)
    Wk = dt_("Wk", [C, 512], BF16)
    Wv = dt_("Wv", [C, C], BF16)
    bv = dt_("bv", [1, C], BF16)
    Wqp = dt_("Wqp", [C, HD], BF16)
    bqp = dt_("bqp", [1, HD], BF16)
    Wkp = dt_("Wkp", [C, HD], BF16)
    bkp = dt_("bkp", [1, HD], BF16)
    Wpb = dt_("Wpb", [PC, H], FP8)
    Wo = dt_("Wo", [C, C], BF16)
    bo = dt_("bo", [1, C], BF16)
    gamB = dt_("gamB", [NO, C])
    betB = dt_("betB", [NO, C])
    id128 = dt_("id128", [128, 128], BF16)
    out = nc.dram_tensor("out", [NO, C], F32, kind="ExternalOutput").ap()

    with tile.TileContext(nc) as tc:
        _kernel(tc, locals())
    return out


def _kernel(tc, t):
    nc = tc.nc
    mm = nc.tensor.matmul
    dma = nc.sync.dma_start

    const = tc.alloc_tile_pool(name="const", bufs=1)
    big = tc.alloc_tile_pool(name="big", bufs=1)

    # ---- weights / constants into SBUF (3 c-tiles side by side) ----
    def load3(name, cols, dt=BF16):
        tl = const.tile([128, 3 * cols], dt, tag=name + "_sb")
        for tt in range(3):
            dma(tl[:, tt * cols:(tt + 1) * cols],
                t[name][tt * 128:(tt + 1) * 128, :])
        return tl

    Wq_sb = load3("Wq", 512)
    Wk_sb = load3("Wk", 512)
    Wv_sb = load3("Wv", C)
    Wo_sb = load3("Wo", C)
    Wqp_sb = load3("Wqp", HD)
    Wkp_sb = load3("Wkp", HD)
    sT_sb = load3("singleT", N)
    sTo_sb = load3("sTo", NO)

    def load(name, shape, src, dt=F32):
        tl = const.tile(list(shape), dt, tag=name)
        dma(tl[:], src)
        return tl

    Wpb_sb = load("Wpb_sb", (PC, H), t["Wpb"], FP8)
    bq_sb = load("bq_sb", (1, 512), t["bq"], BF16)
    bv_sb = load("bv_sb", (1, C), t["bv"], BF16)
    bqp_sb = load("bqp_sb", (1, HD), t["bqp"], BF16)
    bkp_sb = load("bkp_sb", (1, HD), t["bkp"], BF16)
    bo_sb = load("bo_sb", (1, C), t["bo"], BF16)
    id_sb = load("id_sb", (128, 128), t["id128"], BF16)
    gam_sb = load("gam_sb", (NO, C), t["gamB"])
    bet_sb = load("bet_sb", (NO, C), t["betB"])
    so_sb = load("so_sb", (NO, C), t["so"])
    roto_sb = load("roto_sb", (NO, 9), t["roto"])
    transqo_sb = load("transqo_sb", (NO, 3), t["transqo"])
    rot_sb = const.tile([128, MT * 9], F32, tag="rot_sb")
    trans_sb = const.tile([128, MT * 3], F32, tag="trans_sb")
    for mt in range(MT):
        dma(rot_sb[:, mt * 9:(mt + 1) * 9], t["rot9"][mt * 128:(mt + 1) * 128, :])
        dma(trans_sb[:, mt * 3:(mt + 1) * 3], t["trans3"][mt * 128:(mt + 1) * 128, :])

    # ---- pair groups: issue all DMAs up front; bufs throttles concurrency
    # so early groups land before PE needs them ----
    pair = tc.alloc_tile_pool(name="pair", bufs=3)
    pg_tiles = []
    if STAGE != "proj":
        for g in range(MT):
            pg = pair.tile([128, GSZ], FP8, tag="pg")
            dma(pg[:], t["pairT"][:, g * GSZ:(g + 1) * GSZ])
            pg_tiles.append(pg)

    ones96 = const.tile([1, NO], BF16, tag="ones96")
    nc.vector.memset(ones96[:], 1.0)
    ones128 = const.tile([1, 128], BF16, tag="ones128")
    nc.vector.memset(ones128[:], 1.0)

    # ---- persistent slabs ----
    # chem packing: head h -> block h//3 (cols of 768/96), rows 32*(h%3)
    # (PE row-group base must be 0/32/64 -- quadrant 3 is unusable)
    kT_sb = big.tile([128, 4 * N], BF16, tag="kT")
    qT_sb = big.tile([128, 4 * NO], BF16, tag="qT")
    # point packing: head h -> block h//3, rows 32*(h%3), 9 rows
    kgT_sb = big.tile([128, 4 * N], BF16, tag="kgT")
    qgT_sb = big.tile([128, 4 * NO], BF16, tag="qgT")
    # v with interleaved ones-columns: per m-group [128, H*(Ch+1)]
    v_sb = big.tile([128, MT * H * (Ch + 1)], BF16, tag="v")
    k2s_sb = big.tile([128, MT * H], F32, tag="k2s")   # -0.5*SCALE*k2
    VG = H * (Ch + 1)  # 396
    # combined per-head K-tensors for single full-K logit matmuls:
    # rows 0-31 chem k/q, rows 32-40 point kg/qg, rows 41-127 zero.
    # (bf16 FWL matmuls crash when accumulating sub-128-row lhsT at mixed
    # row-group bases; full-K base-0 single matmuls are the safe shape)
    ckT_sb = big.tile([128, H * N], BF16, tag="ckT")
    cqT_sb = big.tile([128, H * NO], BF16, tag="cqT")
    nc.vector.memset(ckT_sb[:], 0.0)
    nc.vector.memset(cqT_sb[:], 0.0)

    nc.vector.memset(
        v_sb[:].rearrange("p (x e) -> p x e", e=Ch + 1)[:, :, Ch:Ch + 1], 1.0)

    # work bufs=6: kp/kg/kgb tile lists span all 6 m-tiles across separate
    # loops -- fewer bufs alias live tiles and deadlock the engine queues
    with tc.tile_pool(name="pro", bufs=3, space="PSUM") as pro, \
         tc.tile_pool(name="work", bufs=6) as work:

        # ---- point projections first so DVE rotations start early ----
        qp_f = work.tile([NO, HD], F32, tag="qp")
        ps = pro.tile([128, 384], F32, tag="ps")
        for tt in range(3):
            mm(ps[:NO, :HD], sTo_sb[:, tt * NO:(tt + 1) * NO],
               Wqp_sb[:, tt * HD:(tt + 1) * HD], start=(tt == 0), stop=False)
        mm(ps[:NO, :HD], ones128[:, :NO], bqp_sb[:], start=False, stop=True)
        nc.vector.tensor_copy(qp_f[:], ps[:NO, :HD])

        kp_tiles = []
        for mt in range(MT):
            ps = pro.tile([128, 384], F32, tag="ps")
            for tt in range(3):
                mm(ps[:, :HD], sT_sb[:, tt * N + mt * 128: tt * N + (mt + 1) * 128],
                   Wkp_sb[:, tt * HD:(tt + 1) * HD], start=(tt == 0), stop=False)
            mm(ps[:, :HD], ones128[:], bkp_sb[:], start=False, stop=True)
            kp = work.tile([128, HD], F32, tag="kp")
            nc.vector.tensor_copy(kp[:], ps[:, :HD])
            kp_tiles.append(kp)

        # ---- rotations (fp32 DVE), overlap with chem projections below ----
        def rotate(dst, src, rsb, roff, tsb, toff, rows):
            dv = dst.rearrange("p (h d j) -> p h d j", d=3, j=3)
            sv = src.rearrange("p (h d i) -> p h d i", d=3, i=3)
            for j in range(3):
                acc = work.tile([rows, 36], F32, tag="rotacc")
                av = acc[:].rearrange("p (h d) -> p h d", d=3)
                nc.vector.tensor_scalar_mul(av, sv[:, :, :, 0],
                                            rsb[:rows, roff + j: roff + j + 1])
                for i in (1, 2):
                    nc.vector.scalar_tensor_tensor(
                        av, sv[:, :, :, i],
                        rsb[:rows, roff + 3 * i + j: roff + 3 * i + j + 1],
                        av, op0=ALU.mult, op1=ALU.add)
                nc.vector.tensor_copy(dv[:, :, :, j], av)
            for d in range(3):
                nc.vector.tensor_scalar_add(dv[:, :, d, :], dv[:, :, d, :],
                                            tsb[:rows, toff + d: toff + d + 1])

        qg_f = work.tile([NO, HD], F32, tag="qg")
        qg_b = work.tile([NO, HD], BF16, tag="qgb")
        rotate(qg_f[:], qp_f[:], roto_sb, 0, transqo_sb, 0, NO)
        nc.vector.tensor_copy(qg_b[:], qg_f[:])
        kg_b_tiles = []
        for mt in range(MT):
            kg_f = work.tile([128, HD], F32, tag="kg")
            rotate(kg_f[:], kp_tiles[mt][:], rot_sb, mt * 9, trans_sb, mt * 3, 128)
            kg_b = work.tile([128, HD], BF16, tag="kgb")
            nc.vector.tensor_copy(kg_b[:], kg_f[:])
            kg_b_tiles.append(kg_b)
            # k2s = -0.5*SCALE*sum_dj kg^2 : [m, h]
            sq = work.tile([128, HD], F32, tag="sq")
            nc.vector.tensor_mul(sq[:], kg_f[:], kg_f[:])
            red = work.tile([128, H], F32, tag="red")
            nc.vector.tensor_reduce(red[:], sq[:].rearrange("p (h e) -> p h e", e=9),
                                    axis=AX.X, op=ALU.add)
            nc.vector.tensor_scalar_mul(k2s_sb[:, mt * H:(mt + 1) * H], red[:],
                                        -0.5 * SCALE)

        # ---- chem projections: kT = (single @ Wk)^T, qT, v ----
        for j in range(4):
            for half in range(2):
                ps = pro.tile([128, 384], F32, tag="ps")
                for tt in range(3):
                    mm(ps[:], Wk_sb[:, tt * 512 + j * 128: tt * 512 + (j + 1) * 128],
                       sT_sb[:, tt * N + half * 384: tt * N + (half + 1) * 384],
                       start=(tt == 0), stop=(tt == 2))
                nc.vector.tensor_copy(
                    kT_sb[:, j * N + half * 384: j * N + (half + 1) * 384], ps[:])
        for j in range(4):
            ps = pro.tile([128, 384], F32, tag="ps")
            for tt in range(3):
                mm(ps[:, :NO], Wq_sb[:, tt * 512 + j * 128: tt * 512 + (j + 1) * 128],
                   sTo_sb[:, tt * NO:(tt + 1) * NO], start=(tt == 0), stop=False)
            mm(ps[:, :NO], bq_sb[0:1, j * 128:(j + 1) * 128], ones96[:],
               start=False, stop=True)
            nc.vector.tensor_copy(qT_sb[:, j * NO:(j + 1) * NO], ps[:, :NO])
        for mt in range(MT):
            ps = pro.tile([128, 384], F32, tag="ps")
            for tt in range(3):
                mm(ps[:], sT_sb[:, tt * N + mt * 128: tt * N + (mt + 1) * 128],
                   Wv_sb[:, tt * C:(tt + 1) * C], start=(tt == 0), stop=False)
            mm(ps[:], ones128[:], bv_sb[:], start=False, stop=True)
            dst = v_sb[:, mt * VG:(mt + 1) * VG] \
                .rearrange("p (h e) -> p h e", e=Ch + 1)[:, :, 0:Ch]
            nc.vector.tensor_copy(dst, ps[:].rearrange("p (h c) -> p h c", c=Ch))

        # ---- point transposes into head-block layout ----
        for b in range(4):
            ps = pro.tile([128, 384], F32, tag="ps")
            for hh in range(3):
                mm(ps[32 * hh:32 * hh + 9, :NO],
                   qg_b[:, (3 * b + hh) * 9:(3 * b + hh + 1) * 9],
                   id_sb[:NO, :NO], start=True, stop=True)
            nc.vector.tensor_copy(qgT_sb[:96, b * NO:(b + 1) * NO], ps[:96, :NO])
        for mt in range(MT):
            for b in range(4):
                ps = pro.tile([128, 384], F32, tag="ps")
                for hh in range(3):
                    mm(ps[32 * hh:32 * hh + 9, :128],
                       kg_b_tiles[mt][:, (3 * b + hh) * 9:(3 * b + hh + 1) * 9],
                       id_sb[:], start=True, stop=True)
                nc.vector.tensor_copy(
                    kgT_sb[:96, b * N + mt * 128: b * N + (mt + 1) * 128],
                    ps[:96, :128])

        # assemble combined slabs (SBUF->SBUF DMA shifts partitions)
        for h in range(H):
            b, tr = h // 3, 32 * (h % 3)
            dma(ckT_sb[0:32, h * N:(h + 1) * N],
                kT_sb[tr:tr + 32, b * N:(b + 1) * N])
            dma(ckT_sb[32:41, h * N:(h + 1) * N],
                kgT_sb[tr:tr + 9, b * N:(b + 1) * N])
            dma(cqT_sb[0:32, h * NO:(h + 1) * NO],
                qT_sb[tr:tr + 32, b * NO:(b + 1) * NO])
            dma(cqT_sb[32:41, h * NO:(h + 1) * NO],
                qgT_sb[tr:tr + 9, b * NO:(b + 1) * NO])

    if STAGE == "proj":
        dbg = const.tile([128, 4 * N], F32, tag="dbg")
        nc.vector.tensor_copy(dbg[:], kgT_sb[:])
        dma(t["out"], dbg[:NO, :C])
        pair.release()
        big.release()
        const.release()
        return

    # ---- attention: m-major pipeline over 6 groups ----
    try:
        _attention(tc, t, locals())
    except _StageExit:
        pass
    pair.release()
    big.release()
    const.release()


class _StageExit(Exception):
    pass


def _attention(tc, t, env):
    nc = tc.nc
    mm = nc.tensor.matmul
    dma = nc.sync.dma_start
    for _k, _v in env.items():
        if _k not in ("tc", "t", "nc", "mm", "dma"):
            globals().setdefault("_ENV_UNUSED", None)
    (pg_tiles, ckT_sb, cqT_sb, v_sb, k2s_sb, Wpb_sb, id_sb,
     so_sb, gam_sb, bet_sb, Wo_sb, bo_sb, ones96) = (
        env["pg_tiles"], env["ckT_sb"], env["cqT_sb"],
        env["v_sb"], env["k2s_sb"], env["Wpb_sb"],
        env["id_sb"], env["so_sb"], env["gam_sb"], env["bet_sb"],
        env["Wo_sb"], env["bo_sb"], env["ones96"])
    VG = H * (Ch + 1)
    with tc.tile_pool(name="pL", bufs=3, space="PSUM") as pL, \
         tc.tile_pool(name="pp", bufs=2, space="PSUM") as pp, \
         tc.tile_pool(name="pacc", bufs=1, space="PSUM") as pacc, \
         tc.tile_pool(name="att", bufs=2) as att:
        av_ps = pacc.tile([128, VG], F32, tag="av")
        E_tiles = []

        def av_mms(g):
            E_sb = E_tiles[g]
            for h in range(H):
                mm(av_ps[:NO, h * (Ch + 1):(h + 1) * (Ch + 1)],
                   E_sb[:, h * NO:(h + 1) * NO],
                   v_sb[:, g * VG + h * (Ch + 1): g * VG + (h + 1) * (Ch + 1)],
                   start=(g == 0), stop=(g == MT - 1))

        for g in range(MT):
            # logit matmuls: 3 L tiles x 4 heads, chem + point accumulation
            L_tiles = [pL.tile([128, 384], F32, tag="L", name=f"L{i}")
                       for i in range(3)]
            for h in range(H if STAGE != "pgemm" else 0):
                Lr = L_tiles[h // 4][:, (h % 4) * NO:(h % 4 + 1) * NO]
                mm(Lr, ckT_sb[:, h * N + g * 128: h * N + (g + 1) * 128],
                   cqT_sb[:, h * NO:(h + 1) * NO], start=True, stop=True)

            if g > 0 and STAGE != "pair":
                av_mms(g - 1)

            # pair GEMM: [m, 12] per n, 3 psum chunks of 32 n's
            pgv = pg_tiles[g][:].rearrange("p (n m) -> p n m", m=128)
            pb_sb = att.tile([128, H * NO], F32, tag="pb")
            for c in range(3 if STAGE != "lmm" else 0):
                ps = pp.tile([128, 384], F32, tag="pps")
                for i in range(32):
                    nsl = pgv[:, c * 32 + i: c * 32 + i + 1, :]
                    mm(ps[:, i * H:(i + 1) * H], nsl, Wpb_sb[:],
                       start=True, stop=True)
                nc.vector.tensor_scalar_mul(
                    pb_sb[:, c * 32 * H:(c + 1) * 32 * H], ps[:], 1.0 / WPB_SCALE)

            # combine + exp: E[m, (h, n)] bf16
            E_sb = att.tile([128, H * NO], BF16, tag="E")
            E_tiles.append(E_sb)
            pbv = pb_sb[:].rearrange("p (n h) -> p h n", h=H)
            for h in range(H):
                Lr = L_tiles[h // 4][:, (h % 4) * NO:(h % 4 + 1) * NO]
                tmp = att.tile([128, NO], F32, tag="tmp")
                if STAGE == "pgemm":
                    nc.vector.tensor_copy(tmp[:], pbv[:, h:h + 1, :])
                elif STAGE == "lmm":
                    nc.vector.tensor_copy(tmp[:], Lr)
                else:
                    nc.vector.tensor_add(tmp[:], Lr, pbv[:, h:h + 1, :])
                nc.scalar.activation(E_sb[:, h * NO:(h + 1) * NO], tmp[:],
                                     ACTF.Exp,
                                     bias=k2s_sb[:, g * H + h: g * H + h + 1])
        if STAGE in ("pair", "pgemm", "lmm"):
            dbg = att.tile([NO, C], F32, tag="dbg")
            nc.vector.tensor_copy(dbg[:], E_tiles[-1][:NO, :C])
            dma(t["out"], dbg[:])
            raise _StageExit
        av_mms(MT - 1)

        # ---- epilogue: divide, out-proj, residual, layernorm ----
        avv = av_ps[:NO, :].rearrange("p (h e) -> p h e", e=Ch + 1)
        rcp = att.tile([NO, H], F32, tag="rcp")
        nc.vector.reciprocal(rcp[:], avv[:, :, Ch:Ch + 1])
        w_sb = att.tile([NO, C], BF16, tag="w")
        for h in range(H):
            nc.vector.tensor_scalar_mul(w_sb[:, h * Ch:(h + 1) * Ch],
                                        avv[:, h:h + 1, 0:Ch], rcp[:, h:h + 1])
        wT_sb = att.tile([128, 3 * NO], BF16, tag="wT")
        for tt in range(3):
            tp = pL.tile([128, 384], F32, tag="L")
            mm(tp[:, :NO], w_sb[:, tt * 128:(tt + 1) * 128], id_sb[:NO, :NO],
               start=True, stop=True)
            nc.vector.tensor_copy(wT_sb[:, tt * NO:(tt + 1) * NO], tp[:, :NO])
        o_ps = pacc.tile([NO, C], F32, tag="o")
        for tt in range(3):
            mm(o_ps[:], wT_sb[:, tt * NO:(tt + 1) * NO], Wo_sb[:, tt * C:(tt + 1) * C],
               start=(tt == 0), stop=False)
        mm(o_ps[:], ones96[:], bo_sb[:], start=False, stop=True)
        x_sb = att.tile([NO, C], F32, tag="x")
        nc.vector.tensor_add(x_sb[:], o_ps[:], so_sb[:])
        mu = att.tile([NO, 1], F32, tag="mu")
        nc.vector.tensor_reduce(mu[:], x_sb[:], axis=AX.X, op=ALU.add)
        nc.vector.tensor_scalar_mul(mu[:], mu[:], 1.0 / C)
        xm = att.tile([NO, C], F32, tag="xm")
        nc.vector.tensor_scalar_sub(xm[:], x_sb[:], mu[:])
        sq = att.tile([NO, C], F32, tag="sqe")
        nc.vector.tensor_mul(sq[:], xm[:], xm[:])
        var = att.tile([NO, 1], F32, tag="var")
        nc.vector.tensor_reduce(var[:], sq[:], axis=AX.X, op=ALU.add)
        epsb = att.tile([NO, 1], F32, tag="epsb")
        nc.vector.memset(epsb[:], EPS)
        std = att.tile([NO, 1], F32, tag="std")
        nc.scalar.activation(std[:], var[:], ACTF.Sqrt, bias=epsb[:], scale=1.0 / C)
        rstd = att.tile([NO, 1], F32, tag="rstd")
        nc.vector.reciprocal(rstd[:], std[:])
        y = att.tile([NO, C], F32, tag="y")
        nc.vector.tensor_scalar_mul(y[:], xm[:], rstd[:])
        nc.vector.tensor_mul(y[:], y[:], gam_sb[:])
        nc.vector.tensor_add(y[:], y[:], bet_sb[:])
        dma(t["out"], y[:])


_CACHE = {}


def _get_program():
    if "nc" not in _CACHE:
        nc = bacc.Bacc("TRN2", target_bir_lowering=False, debug=False,
                       num_devices=NCORES)
        _build(nc)
        nc.compile()
        _CACHE["nc"] = nc
    return _CACHE["nc"]


def _pad_heads(W):
    # scatter head h (32 cols) to col 128*(h//3) + 32*(h%3) of a 512-wide buffer
    out = np.zeros(W.shape[:-1] + (512,), np.float32)
    for h in range(H):
        c0 = 128 * (h // 3) + 32 * (h % 3)
        out[..., c0:c0 + Ch] = W[..., h * Ch:(h + 1) * Ch]
    return out


def make_in_maps(single, pair, rot, trans, Wq, bq, Wk, bk, Wv, bv, Wpb, bpb,
                 Wqp, bqp, Wkp, bkp, Wo, bo, gamma, beta):
    f = lambda a: np.ascontiguousarray(np.asarray(a), dtype=np.float32)
    bf = lambda a: np.ascontiguousarray(np.asarray(a, dtype=np.float32)
                                        .astype(ml_dtypes.bfloat16))
    s = f(single)[0]
    common = {
        "singleT": bf(s.T),
        "rot9": f(rot)[0].reshape(N, 9),
        "trans3": f(trans)[0],
        "Wq": bf(_pad_heads(f(Wq) * SCALE)),
        "bq": bf(_pad_heads((f(bq) * SCALE).reshape(1, C))),
        "Wk": bf(_pad_heads(f(Wk))),
        "Wv": bf(Wv), "bv": bf(f(bv).reshape(1, C)),
        "Wqp": bf(f(Wqp) * SCALE), "bqp": bf((f(bqp) * SCALE).reshape(1, HD)),
        "Wkp": bf(Wkp), "bkp": bf(f(bkp).reshape(1, HD)),
        "Wpb": np.ascontiguousarray(
            (f(Wpb) * WPB_SCALE).astype(ml_dtypes.float8_e4m3)),
        "Wo": bf(Wo), "bo": bf(f(bo).reshape(1, C)),
        "gamB": np.ascontiguousarray(np.broadcast_to(f(gamma), (NO, C))),
        "betB": np.ascontiguousarray(np.broadcast_to(f(beta), (NO, C))),
        "id128": np.eye(128, dtype=ml_dtypes.bfloat16),
    }
    pr = f(pair)[0]
    tr = f(trans)[0] * SCALE
    ro = f(rot)[0].reshape(N, 9)
    in_maps = []
    for c in range(NCORES):
        lo, hi = c * NO, (c + 1) * NO
        m = dict(common)
        m["sTo"] = bf(s[lo:hi].T)
        m["so"] = np.ascontiguousarray(s[lo:hi])
        # [n, m, pc] -> [pc, g, n, m] so each group's lhsT columns (m) are
        # contiguous (FWL needs contiguous 128-col weight reads)
        pq = pr[lo:hi].transpose(2, 1, 0).reshape(PC, MT, 128, NO) \
            .transpose(0, 1, 3, 2).reshape(PC, MT * GSZ)
        m["pairT"] = np.ascontiguousarray(pq.astype(ml_dtypes.float8_e4m3))
        m["roto"] = np.ascontiguousarray(ro[lo:hi])
        m["transqo"] = np.ascontiguousarray(tr[lo:hi])
        in_maps.append(m)
    return in_maps


def run(in_maps, **kwargs):
    nc = _get_program()
    return run_bass_kernel_spmd(nc, in_maps, core_ids=list(range(NCORES)), **kwargs)


def kernel(**inputs):
    res = run(make_in_maps(**inputs))
    out = np.concatenate([res.results[c]["out"] for c in range(NCORES)], axis=0)
    return out.reshape(B, N, C).astype(np.float32)


# revision 28
# speedup vs baseline: 4.3025x; 1.4627x over previous
"""Invariant Point Attention kernel for Trainium2, 8-core SPMD.

Strategy: sequence-parallel over the query axis n (96 rows/core), m-major
pipeline over 6 key-tiles of 128. Each core computes full k/v/k_pts from
`single` (replicated, tiny), its own q rows, and streams its [96, 768, 128]
pair slice (host-transposed, fp8) group by group, overlapping DMA with PE.

Perf notes vs the fp32 baseline (460us):
  - all matmul operands bf16 (fp8 for pair) -> FWL weight loads + 1
    cycle/row streaming instead of fp32's 4;
  - softmax denominator fused into the attn@v matmul via ones-columns
    interleaved in v;
  - chem (q.k), point (qg.kg) and k2 terms fold into ONE full-K matmul
    per (m-group, head) via combined K-tensors (rows 0-31 chem, 32-40
    point, 41 k2s^T/ones, rest zero). Also dodges a HW hang: bf16 FWL
    matmuls accumulating sub-128-row lhsT at mixed row-group bases crash;
  - DMA issue costs ~650ns each on the serial sync queue, so DMA count
    is minimized: constants arrive as 3 host-packed blocks, and the
    combined K-tensors are assembled via a DRAM bounce (5 verbatim
    writes + 5 layout-transforming reads) instead of 60 small copies;
  - one wide exp per m-group (ACT fixed cost ~300ns/instr);
  - elementwise work spread across Vector/Scalar/GpSimd engines.

Math notes vs the reference:
  - terms constant along the softmax axis m cancel exactly (q2, bk, bpb)
    and are dropped;
  - SCALE is folded into Wq/bq, Wqp/bqp and the q-side trans on the host;
  - Wpb is scaled x64 on the host to stay out of fp8 subnormals; the pair
    psum copy divides by 64 (ACT scaled-copy);
  - softmax runs without max-subtraction (logits are O(10), exp is safe
    in fp32); the denominator is applied after attn@v by linearity;
  - rotation + k2 stay fp32; only matmul operands are quantized.
"""

import os
import sys

for p in ("/opt/trn_rl_repo", "/opt/trn_rl_repo/concourse"):
    if p not in sys.path:
        sys.path.append(p)

import numpy as np
import ml_dtypes

import concourse.bass as bass
import concourse.tile as tile
from concourse import bacc, mybir
from concourse.bass_utils import run_bass_kernel_spmd

F32 = mybir.dt.float32
BF16 = mybir.dt.bfloat16
FP8 = mybir.dt.float8e4
AX = mybir.AxisListType
ALU = mybir.AluOpType
ACTF = mybir.ActivationFunctionType

B, N, C, PC, H, P = 1, 768, 384, 128, 12, 3
Ch = C // H            # 32
HD = H * P * P         # 108
SCALE = Ch ** -0.5
EPS = 1e-5
NCORES = 8
NO = N // NCORES       # 96 own query rows per core
MT = N // 128          # 6 m-groups
GSZ = NO * 128         # pair elems per partition per m-group
WPB_SCALE = 64.0       # fp8 subnormal dodge for Wpb
VG = H * (Ch + 1)      # 396: v block incl ones-columns

# common bf16 const block column offsets
_O_ST = 0                      # singleT   [128, 3*768]
_O_WQP = _O_ST + 3 * N         # Wqp       [128, 3*108]
_O_WKP = _O_WQP + 3 * HD       # Wkp
_O_BQ = _O_WKP + 3 * HD        # row-0: bq [1, 384]
_O_BV = _O_BQ + C
_O_BQP = _O_BV + C
_O_BKP = _O_BQP + HD
_O_BO = _O_BKP + HD
_O_ONESROW = _O_BO + C         # row-0 ones [1, H*NO]
_O_ONES96 = _O_ONESROW + H * NO
_O_ONES128 = _O_ONES96 + NO
_SPLIT1 = _O_ONES128 + 128
_O_WK = _SPLIT1                # Wk [128, 3*384]
_O_WQ = _O_WK + 3 * C
_SPLIT2 = _O_WQ + 3 * C
_O_WV = _SPLIT2
_O_WO = _O_WV + 3 * C
_O_ID = _O_WO + 3 * C
_CBF_COLS = _O_ID + 128
# common f32 block
_F_ROT = 0                     # [128, 6*9]
_F_TRANS = _F_ROT + MT * 9
_F_GAM = _F_TRANS + MT * 3     # rows 0-95
_F_BET = _F_GAM + C
_CF_COLS = _F_BET + C
# per-core f32 block
_P_SO = 0                      # rows 0-95
_P_ROTO = _P_SO + C
_P_TQO = _P_ROTO + 9
_CP_COLS = _P_TQO + 3

STAGE = os.environ.get("K_STAGE", "full")


def _build(nc):
    def dt_(name, shape, dt=F32):
        return nc.dram_tensor(name, shape, dt, kind="ExternalInput").ap()

    CBF = dt_("CBF", [128, _CBF_COLS], BF16)
    CF = dt_("CF", [128, _CF_COLS])
    CPo = dt_("CPo", [128, _CP_COLS])
    sTo = dt_("sTo", [C, NO], BF16)
    Wpb = dt_("Wpb", [PC, H], FP8)
    pairT = dt_("pairT", [PC, MT * GSZ], FP8)
    out = nc.dram_tensor("out", [NO, C], F32, kind="ExternalOutput").ap()

    with tile.TileContext(nc) as tc:
        _kernel(tc, locals())
    return out


def _kernel(tc, t):
    nc = tc.nc
    mm = nc.tensor.matmul
    dma = nc.sync.dma_start
    scopy = nc.scalar.copy          # ACT-engine cast/copy
    vcopy = nc.vector.tensor_copy

    const = tc.alloc_tile_pool(name="const", bufs=1)
    big = tc.alloc_tile_pool(name="big", bufs=1)

    # ---- constants: 3 split DMAs (parallel queues), then slices ----
    CBF_sb = const.tile([128, _CBF_COLS], BF16, tag="CBF")
    dma(CBF_sb[:, 0:_SPLIT1], t["CBF"][:, 0:_SPLIT1])
    dma(CBF_sb[:, _SPLIT1:_SPLIT2], t["CBF"][:, _SPLIT1:_SPLIT2])
    dma(CBF_sb[:, _SPLIT2:], t["CBF"][:, _SPLIT2:])
    CF_sb = const.tile([128, _CF_COLS], F32, tag="CF")
    dma(CF_sb[:], t["CF"])
    CPo_sb = const.tile([128, _CP_COLS], F32, tag="CPo")
    dma(CPo_sb[:], t["CPo"])
    sTo_sb = const.tile([128, 3 * NO], BF16, tag="sTo_sb")
    for tt in range(3):
        dma(sTo_sb[:, tt * NO:(tt + 1) * NO], t["sTo"][tt * 128:(tt + 1) * 128, :])
    Wpb_sb = const.tile([PC, H], FP8, tag="Wpb_sb")
    dma(Wpb_sb[:], t["Wpb"])

    sT_sb = CBF_sb[:, _O_ST:_O_ST + 3 * N]
    Wqp_sb = CBF_sb[:, _O_WQP:_O_WQP + 3 * HD]
    Wkp_sb = CBF_sb[:, _O_WKP:_O_WKP + 3 * HD]
    bq_sb = CBF_sb[0:1, _O_BQ:_O_BQ + C]
    bv_sb = CBF_sb[0:1, _O_BV:_O_BV + C]
    bqp_sb = CBF_sb[0:1, _O_BQP:_O_BQP + HD]
    bkp_sb = CBF_sb[0:1, _O_BKP:_O_BKP + HD]
    bo_sb = CBF_sb[0:1, _O_BO:_O_BO + C]
    ones_row = CBF_sb[0:1, _O_ONESROW:_O_ONESROW + H * NO]
    ones96 = CBF_sb[0:1, _O_ONES96:_O_ONES96 + NO]
    ones128 = CBF_sb[0:1, _O_ONES128:_O_ONES128 + 128]
    Wk_sb = CBF_sb[:, _O_WK:_O_WK + 3 * C]
    Wq_sb = CBF_sb[:, _O_WQ:_O_WQ + 3 * C]
    Wv_sb = CBF_sb[:, _O_WV:_O_WV + 3 * C]
    Wo_sb = CBF_sb[:, _O_WO:_O_WO + 3 * C]
    id_sb = CBF_sb[:, _O_ID:_O_ID + 128]
    rot_sb = CF_sb[:, _F_ROT:_F_ROT + MT * 9]
    trans_sb = CF_sb[:, _F_TRANS:_F_TRANS + MT * 3]
    gam_sb = CF_sb[0:NO, _F_GAM:_F_GAM + C]
    bet_sb = CF_sb[0:NO, _F_BET:_F_BET + C]
    so_sb = CPo_sb[0:NO, _P_SO:_P_SO + C]
    roto_sb = CPo_sb[0:NO, _P_ROTO:_P_ROTO + 9]
    transqo_sb = CPo_sb[0:NO, _P_TQO:_P_TQO + 3]

    # ---- pair groups: issue all DMAs up front; bufs throttles concurrency
    # so early groups land before PE needs them ----
    pair = tc.alloc_tile_pool(name="pair", bufs=3)
    pg_tiles = []
    if STAGE != "proj":
        for g in range(MT):
            pg = pair.tile([128, GSZ], FP8, tag="pg")
            dma(pg[:], t["pairT"][:, g * GSZ:(g + 1) * GSZ])
            pg_tiles.append(pg)

    # ---- persistent slabs ----
    # chem staging: natural packing, head h -> block h//4, rows 32*(h%4)
    kT_sb = big.tile([128, 3 * N], BF16, tag="kT")
    qT_sb = big.tile([128, 3 * NO], BF16, tag="qT")
    # point staging: head h -> block h//3, rows 32*(h%3), 9 rows
    kgT_sb = big.tile([128, 4 * N], BF16, tag="kgT")
    qgT_sb = big.tile([128, 4 * NO], BF16, tag="qgT")
    # v with interleaved ones-columns: per m-group [128, H*(Ch+1)]
    v_sb = big.tile([128, MT * VG], BF16, tag="v")
    k2s_sb = big.tile([128, MT * H], F32, tag="k2s")   # -0.5*SCALE*k2, h-major
    k2s_b = big.tile([128, MT * H], BF16, tag="k2sb")
    kt2_b = big.tile([MT * H, 128], BF16, tag="kt2")   # k2s^T rows (h*MT+mt)
    # combined per-head K-tensors for single full-K logit matmuls
    ckT_sb = big.tile([128, H * N], BF16, tag="ckT")
    cqT_sb = big.tile([128, H * NO], BF16, tag="cqT")
    nc.gpsimd.memset(ckT_sb[:], 0.0)
    nc.gpsimd.memset(cqT_sb[:], 0.0)
    # E: two manual buffers, padded to 128 cols/head for FWL; zero the pads
    E_bufs = [big.tile([128, H * 128], BF16, tag="E0", name="E0"),
              big.tile([128, H * 128], BF16, tag="E1", name="E1")]
    for eb in E_bufs:
        nc.gpsimd.memset(
            eb[:].rearrange("p (h m) -> p h m", m=128)[:, :, NO:128], 0.0)
    nc.gpsimd.memset(
        v_sb[:].rearrange("p (x e) -> p x e", e=Ch + 1)[:, :, Ch:Ch + 1], 1.0)

    # DRAM bounce for combined-slab assembly (partition-shifting gathers)
    bounce = tc.alloc_tile_pool(name="bounce", bufs=1, space="DRAM")
    dk = bounce.tile([128, 3 * N], BF16, tag="dk")
    dkg = bounce.tile([96, 4 * N], BF16, tag="dkg")
    dq = bounce.tile([128, 3 * NO], BF16, tag="dq")
    dqg = bounce.tile([96, 4 * NO], BF16, tag="dqg")
    dk2 = bounce.tile([MT * H, 128], BF16, tag="dk2")

    with tc.tile_pool(name="pro", bufs=3, space="PSUM") as pro, \
         tc.tile_pool(name="work", bufs=6) as work:

        # ---- point projections first so DVE rotations start early ----
        qp_f = work.tile([NO, HD], F32, tag="qp")
        ps = pro.tile([128, 384], F32, tag="ps")
        for tt in range(3):
            mm(ps[:NO, :HD], sTo_sb[:, tt * NO:(tt + 1) * NO],
               Wqp_sb[:, tt * HD:(tt + 1) * HD], start=(tt == 0), stop=False)
        mm(ps[:NO, :HD], ones128[:, :NO], bqp_sb[:], start=False, stop=True)
        scopy(qp_f[:], ps[:NO, :HD])

        kp_tiles = []
        for mt in range(MT):
            ps = pro.tile([128, 384], F32, tag="ps")
            for tt in range(3):
                mm(ps[:, :HD], sT_sb[:, tt * N + mt * 128: tt * N + (mt + 1) * 128],
                   Wkp_sb[:, tt * HD:(tt + 1) * HD], start=(tt == 0), stop=False)
            mm(ps[:, :HD], ones128[:], bkp_sb[:], start=False, stop=True)
            kp = work.tile([128, HD], F32, tag="kp")
            scopy(kp[:], ps[:, :HD])
            kp_tiles.append(kp)

        # ---- rotations (fp32 DVE) ----
        def rotate(dst, src, rsb, roff, tsb, toff, rows):
            dv = dst.rearrange("p (h d j) -> p h d j", d=3, j=3)
            sv = src.rearrange("p (h d i) -> p h d i", d=3, i=3)
            for j in range(3):
                acc = work.tile([rows, 36], F32, tag="rotacc")
                av = acc[:].rearrange("p (h d) -> p h d", d=3)
                nc.vector.tensor_scalar_mul(av, sv[:, :, :, 0],
                                            rsb[:rows, roff + j: roff + j + 1])
                for i in (1, 2):
                    nc.vector.scalar_tensor_tensor(
                        av, sv[:, :, :, i],
                        rsb[:rows, roff + 3 * i + j: roff + 3 * i + j + 1],
                        av, op0=ALU.mult, op1=ALU.add)
                nc.vector.tensor_copy(dv[:, :, :, j], av)
            for d in range(3):
                nc.vector.tensor_scalar_add(dv[:, :, d, :], dv[:, :, d, :],
                                            tsb[:rows, toff + d: toff + d + 1])

        qg_f = work.tile([NO, HD], F32, tag="qg")
        qg_b = work.tile([NO, HD], BF16, tag="qgb")
        rotate(qg_f[:], qp_f[:], roto_sb, 0, transqo_sb, 0, NO)
        scopy(qg_b[:], qg_f[:])
        kg_b_tiles = []
        for mt in range(MT):
            kg_f = work.tile([128, HD], F32, tag="kg")
            rotate(kg_f[:], kp_tiles[mt][:], rot_sb, mt * 9, trans_sb, mt * 3, 128)
            kg_b = work.tile([128, HD], BF16, tag="kgb")
            scopy(kg_b[:], kg_f[:])
            kg_b_tiles.append(kg_b)
            # k2s = -0.5*SCALE*sum_dj kg^2 : [m, h] stored h-major (h*MT+mt)
            sq = work.tile([128, HD], F32, tag="sq")
            nc.vector.tensor_mul(sq[:], kg_f[:], kg_f[:])
            red = work.tile([128, H], F32, tag="red")
            nc.vector.tensor_reduce(red[:], sq[:].rearrange("p (h e) -> p h e", e=9),
                                    axis=AX.X, op=ALU.add)
            nc.vector.tensor_scalar_mul(
                k2s_sb[:].rearrange("p (h mt) -> p h mt", mt=MT)[:, :, mt:mt + 1],
                red[:], -0.5 * SCALE)
        vcopy(k2s_b[:], k2s_sb[:])

        # ---- chem projections: kT = (single @ Wk)^T, qT, v ----
        for j in range(3):
            for half in range(2):
                ps = pro.tile([128, 384], F32, tag="ps")
                for tt in range(3):
                    mm(ps[:], Wk_sb[:, tt * C + j * 128: tt * C + (j + 1) * 128],
                       sT_sb[:, tt * N + half * 384: tt * N + (half + 1) * 384],
                       start=(tt == 0), stop=(tt == 2))
                scopy(kT_sb[:, j * N + half * 384: j * N + (half + 1) * 384], ps[:])
        for j in range(3):
            ps = pro.tile([128, 384], F32, tag="ps")
            for tt in range(3):
                mm(ps[:, :NO], Wq_sb[:, tt * C + j * 128: tt * C + (j + 1) * 128],
                   sTo_sb[:, tt * NO:(tt + 1) * NO], start=(tt == 0), stop=False)
            mm(ps[:, :NO], bq_sb[0:1, j * 128:(j + 1) * 128], ones96[:],
               start=False, stop=True)
            scopy(qT_sb[:, j * NO:(j + 1) * NO], ps[:, :NO])
        for mt in range(MT):
            ps = pro.tile([128, 384], F32, tag="ps")
            for tt in range(3):
                mm(ps[:], sT_sb[:, tt * N + mt * 128: tt * N + (mt + 1) * 128],
                   Wv_sb[:, tt * C:(tt + 1) * C], start=(tt == 0), stop=False)
            mm(ps[:], ones128[:], bv_sb[:], start=False, stop=True)
            dst = v_sb[:, mt * VG:(mt + 1) * VG] \
                .rearrange("p (h e) -> p h e", e=Ch + 1)[:, :, 0:Ch]
            scopy(dst, ps[:].rearrange("p (h c) -> p h c", c=Ch))

        # ---- point transposes into head-block layout ----
        for b in range(4):
            ps = pro.tile([128, 384], F32, tag="ps")
            for hh in range(3):
                mm(ps[32 * hh:32 * hh + 9, :NO],
                   qg_b[:, (3 * b + hh) * 9:(3 * b + hh + 1) * 9],
                   id_sb[:NO, :NO], start=True, stop=True)
            vcopy(qgT_sb[:96, b * NO:(b + 1) * NO], ps[:96, :NO])
        for mt in range(MT):
            for b in range(4):
                ps = pro.tile([128, 384], F32, tag="ps")
                for hh in range(3):
                    mm(ps[32 * hh:32 * hh + 9, :128],
                       kg_b_tiles[mt][:, (3 * b + hh) * 9:(3 * b + hh + 1) * 9],
                       id_sb[:], start=True, stop=True)
                vcopy(kgT_sb[:96, b * N + mt * 128: b * N + (mt + 1) * 128],
                      ps[:96, :128])

        # ---- k2s^T via one PE transpose: row (h*MT+mt), col m-in-tile ----
        ps = pro.tile([128, 384], F32, tag="ps")
        mm(ps[0:MT * H, :128], k2s_b[:], id_sb[:], start=True, stop=True)
        vcopy(kt2_b[:], ps[0:MT * H, :128])

        # ---- assemble combined slabs via DRAM bounce ----
        dma(dk[:], kT_sb[:])
        dma(dkg[:], kgT_sb[0:96, :])
        dma(dq[:], qT_sb[:])
        dma(dqg[:], qgT_sb[0:96, :])
        dma(dk2[:], kt2_b[:])
        # chem: head h = 4j+i lives at staging rows 32i, block j
        dma(ckT_sb[0:32, :].rearrange("p (h m) -> p h m", m=N),
            dk[:].rearrange("(i p) (j m) -> p j i m", i=4, m=N))
        dma(cqT_sb[0:32, :].rearrange("p (h n) -> p h n", n=NO),
            dq[:].rearrange("(i p) (j n) -> p j i n", i=4, n=NO))
        # point: head h = 3b+hh lives at staging rows 32hh (9 used), block b
        dma(ckT_sb[32:41, :].rearrange("p (h m) -> p h m", m=N),
            dkg[:].rearrange("(hh x) (b m) -> x b hh m", hh=3, m=N)[0:9])
        dma(cqT_sb[32:41, :].rearrange("p (h n) -> p h n", n=NO),
            dqg[:].rearrange("(hh x) (b n) -> x b hh n", hh=3, n=NO)[0:9])
        dma(ckT_sb[41:42, :].rearrange("p (h mt m) -> p h mt m", mt=MT, m=128),
            dk2[:].rearrange("(h mt) m -> h mt m", mt=MT))
        dma(cqT_sb[41:42, :], ones_row)

    if STAGE == "proj":
        dbg = const.tile([128, C], F32, tag="dbg")
        nc.vector.tensor_copy(dbg[:], ckT_sb[:, :C])
        dma(t["out"], dbg[:NO, :])
        bounce.release()
        pair.release()
        big.release()
        const.release()
        return

    try:
        _attention(tc, t, locals())
    except _StageExit:
        pass
    bounce.release()
    pair.release()
    big.release()
    const.release()


class _StageExit(Exception):
    pass


def _attention(tc, t, env):
    nc = tc.nc
    mm = nc.tensor.matmul
    dma = nc.sync.dma_start
    (pg_tiles, ckT_sb, cqT_sb, v_sb, E_bufs, Wpb_sb, id_sb,
     so_sb, gam_sb, bet_sb, Wo_sb, bo_sb, ones96) = (
        env["pg_tiles"], env["ckT_sb"], env["cqT_sb"],
        env["v_sb"], env["E_bufs"], env["Wpb_sb"],
        env["id_sb"], env["so_sb"], env["gam_sb"], env["bet_sb"],
        env["Wo_sb"], env["bo_sb"], env["ones96"])

    with tc.tile_pool(name="pL", bufs=3, space="PSUM") as pL, \
         tc.tile_pool(name="pp", bufs=2, space="PSUM") as pp, \
         tc.tile_pool(name="pacc", bufs=1, space="PSUM") as pacc, \
         tc.tile_pool(name="att", bufs=2) as att:
        av_ps = pacc.tile([128, VG], F32, tag="av")

        def av_mms(g):
            E_sb = E_bufs[g % 2]
            for h in range(H):
                mm(av_ps[:, h * (Ch + 1):(h + 1) * (Ch + 1)],
                   E_sb[:, h * 128:(h + 1) * 128],
                   v_sb[:, g * VG + h * (Ch + 1): g * VG + (h + 1) * (Ch + 1)],
                   start=(g == 0), stop=(g == MT - 1))

        for g in range(MT):
            # logit matmuls: one full-K mm per head (chem+point+k2s)
            L_tiles = [pL.tile([128, 384], F32, tag="L", name=f"L{i}")
                       for i in range(3)]
            for h in range(H):
                Lr = L_tiles[h // 4][:, (h % 4) * NO:(h % 4 + 1) * NO]
                mm(Lr, ckT_sb[:, h * N + g * 128: h * N + (g + 1) * 128],
                   cqT_sb[:, h * NO:(h + 1) * NO], start=True, stop=True)

            if g > 0:
                av_mms(g - 1)

            # pair GEMM: [m, 12] per n, 3 psum chunks of 32 n's
            pgv = pg_tiles[g][:].rearrange("p (n m) -> p n m", m=128)
            pb_sb = att.tile([128, H * NO], F32, tag="pb")
            for c in range(3):
                ps = pp.tile([128, 384], F32, tag="pps")
                for i in range(32):
                    nsl = pgv[:, c * 32 + i: c * 32 + i + 1, :]
                    mm(ps[:, i * H:(i + 1) * H], nsl, Wpb_sb[:],
                       start=True, stop=True)
                nc.scalar.activation(pb_sb[:, c * 32 * H:(c + 1) * 32 * H],
                                     ps[:], ACTF.Copy, scale=1.0 / WPB_SCALE)

            # combine (3 wide adds) + one wide exp into padded E
            E_sb = E_bufs[g % 2]
            tmpE = att.tile([128, H * NO], F32, tag="tmpE")
            pbv = pb_sb[:].rearrange("p (n h) -> p h n", h=H)
            for tl in range(3):
                nc.vector.tensor_add(tmpE[:, tl * 4 * NO:(tl + 1) * 4 * NO],
                                     L_tiles[tl][:], pbv[:, 4 * tl:4 * tl + 4, :])
            nc.scalar.activation(
                E_sb[:].rearrange("p (h m) -> p h m", m=128)[:, :, 0:NO],
                tmpE[:], ACTF.Exp)
        av_mms(MT - 1)

        # ---- epilogue: divide, out-proj, residual, layernorm ----
        avv = av_ps[:NO, :].rearrange("p (h e) -> p h e", e=Ch + 1)
        rcp = att.tile([NO, H], F32, tag="rcp")
        nc.vector.reciprocal(rcp[:], avv[:, :, Ch:Ch + 1])
        w_sb = att.tile([NO, C], BF16, tag="w")
        for h in range(H):
            nc.vector.tensor_scalar_mul(w_sb[:, h * Ch:(h + 1) * Ch],
                                        avv[:, h:h + 1, 0:Ch], rcp[:, h:h + 1])
        wT_sb = att.tile([128, 3 * NO], BF16, tag="wT")
        for tt in range(3):
            tp = pL.tile([128, 384], F32, tag="L")
            mm(tp[:, :NO], w_sb[:, tt * 128:(tt + 1) * 128], id_sb[:NO, :NO],
               start=True, stop=True)
            nc.vector.tensor_copy(wT_sb[:, tt * NO:(tt + 1) * NO], tp[:, :NO])
        o_ps = pacc.tile([NO, C], F32, tag="o")
        for tt in range(3):
            mm(o_ps[:], wT_sb[:, tt * NO:(tt + 1) * NO], Wo_sb[:, tt * C:(tt + 1) * C],
               start=(tt == 0), stop=False)
        mm(o_ps[:], ones96[:], bo_sb[:], start=False, stop=True)
        x_sb = att.tile([NO, C], F32, tag="x")
        nc.vector.tensor_add(x_sb[:], o_ps[:], so_sb)
        mu = att.tile([NO, 1], F32, tag="mu")
        nc.vector.tensor_reduce(mu[:], x_sb[:], axis=AX.X, op=ALU.add)
        nc.vector.tensor_scalar_mul(mu[:], mu[:], 1.0 / C)
        xm = att.tile([NO, C], F32, tag="xm")
        nc.vector.tensor_scalar_sub(xm[:], x_sb[:], mu[:])
        sq = att.tile([NO, C], F32, tag="sqe")
        nc.vector.tensor_mul(sq[:], xm[:], xm[:])
        var = att.tile([NO, 1], F32, tag="var")
        nc.vector.tensor_reduce(var[:], sq[:], axis=AX.X, op=ALU.add)
        epsb = att.tile([NO, 1], F32, tag="epsb")
        nc.vector.memset(epsb[:], EPS)
        std = att.tile([NO, 1], F32, tag="std")
        nc.scalar.activation(std[:], var[:], ACTF.Sqrt, bias=epsb[:], scale=1.0 / C)
        rstd = att.tile([NO, 1], F32, tag="rstd")
        nc.vector.reciprocal(rstd[:], std[:])
        y = att.tile([NO, C], F32, tag="y")
        nc.vector.tensor_scalar_mul(y[:], xm[:], rstd[:])
        nc.vector.tensor_mul(y[:], y[:], gam_sb)
        nc.vector.tensor_add(y[:], y[:], bet_sb)
        dma(t["out"], y[:])


_CACHE = {}


def _get_program():
    if "nc" not in _CACHE:
        nc = bacc.Bacc("TRN2", target_bir_lowering=False, debug=False,
                       num_devices=NCORES)
        _build(nc)
        nc.compile()
        _CACHE["nc"] = nc
    return _CACHE["nc"]


def make_in_maps(single, pair, rot, trans, Wq, bq, Wk, bk, Wv, bv, Wpb, bpb,
                 Wqp, bqp, Wkp, bkp, Wo, bo, gamma, beta):
    f = lambda a: np.ascontiguousarray(np.asarray(a), dtype=np.float32)
    b16 = ml_dtypes.bfloat16
    s = f(single)[0]

    cbf = np.zeros((128, _CBF_COLS), b16)

    def put3(off, W, cols):
        Wb = np.asarray(W, np.float32).astype(b16)
        for tt in range(3):
            cbf[:, off + tt * cols:off + (tt + 1) * cols] = \
                Wb[tt * 128:(tt + 1) * 128]

    put3(_O_ST, s.T, N)
    put3(_O_WQP, f(Wqp) * SCALE, HD)
    put3(_O_WKP, f(Wkp), HD)
    cbf[0, _O_BQ:_O_BQ + C] = (f(bq) * SCALE).astype(b16)
    cbf[0, _O_BV:_O_BV + C] = f(bv).astype(b16)
    cbf[0, _O_BQP:_O_BQP + HD] = (f(bqp) * SCALE).astype(b16)
    cbf[0, _O_BKP:_O_BKP + HD] = f(bkp).astype(b16)
    cbf[0, _O_BO:_O_BO + C] = f(bo).astype(b16)
    cbf[0, _O_ONESROW:_O_ONES128 + 128] = b16(1.0)
    put3(_O_WK, f(Wk), C)
    put3(_O_WQ, f(Wq) * SCALE, C)
    put3(_O_WV, f(Wv), C)
    put3(_O_WO, f(Wo), C)
    cbf[:, _O_ID:_O_ID + 128] = np.eye(128, dtype=b16)

    cf = np.zeros((128, _CF_COLS), np.float32)
    ro = f(rot)[0].reshape(N, 9)
    trf = f(trans)[0]
    for mt in range(MT):
        cf[:, _F_ROT + mt * 9:_F_ROT + (mt + 1) * 9] = ro[mt * 128:(mt + 1) * 128]
        cf[:, _F_TRANS + mt * 3:_F_TRANS + (mt + 1) * 3] = \
            trf[mt * 128:(mt + 1) * 128]
    cf[0:NO, _F_GAM:_F_GAM + C] = np.broadcast_to(f(gamma), (NO, C))
    cf[0:NO, _F_BET:_F_BET + C] = np.broadcast_to(f(beta), (NO, C))

    common = {
        "CBF": cbf,
        "CF": cf,
        "Wpb": np.ascontiguousarray(
            (f(Wpb) * WPB_SCALE).astype(ml_dtypes.float8_e4m3)),
    }
    pr = f(pair)[0]
    trs = trf * SCALE
    in_maps = []
    for c in range(NCORES):
        lo, hi = c * NO, (c + 1) * NO
        m = dict(common)
        cp = np.zeros((128, _CP_COLS), np.float32)
        cp[0:NO, _P_SO:_P_SO + C] = s[lo:hi]
        cp[0:NO, _P_ROTO:_P_ROTO + 9] = ro[lo:hi]
        cp[0:NO, _P_TQO:_P_TQO + 3] = trs[lo:hi]
        m["CPo"] = cp
        m["sTo"] = np.ascontiguousarray(s[lo:hi].T.astype(b16))
        # [n, m, pc] -> [pc, g, n, m] so each group's lhsT columns (m) are
        # contiguous (FWL needs contiguous 128-col weight reads)
        pq = pr[lo:hi].transpose(2, 1, 0).reshape(PC, MT, 128, NO) \
            .transpose(0, 1, 3, 2).reshape(PC, MT * GSZ)
        m["pairT"] = np.ascontiguousarray(pq.astype(ml_dtypes.float8_e4m3))
        in_maps.append(m)
    return in_maps


def run(in_maps, **kwargs):
    nc = _get_program()
    return run_bass_kernel_spmd(nc, in_maps, core_ids=list(range(NCORES)), **kwargs)


def kernel(**inputs):
    res = run(make_in_maps(**inputs))
    out = np.concatenate([res.results[c]["out"] for c in range(NCORES)], axis=0)
    return out.reshape(B, N, C).astype(np.float32)
